# revision 56
# baseline (speedup 1.0000x reference)
"""Trainium2 Bass kernel for the DMP-rollout Net (nn_Net_60567628808344).

Math
----
The reference integrates, per row r of p = (x*scale).reshape(-1, 27):
    y0 = p[:,0], goal = p[:,1], w = p[:,2:]
    cx_j = (1 - A_X*DT/TAU)^j                     (data independent)
    psi_j = exp(-0.5 (cx_j - c)^2 / sigma2)       (data independent)
    state update is LINEAR:  s_j = M s_{j-1} + [0; k*(Az*Bz*goal + F_j)]
    with F_j = (w @ psi_j) * cx_j * (goal-y0) / sum(psi_j)
So the whole 301-step rollout collapses to a closed form
    out[r, i] = A[i]*y0_r + B[i]*goal_r + (goal_r - y0_r) * (w_r @ H[i, :])
with constant A,B (301,), H (301, 25) precomputed in float64 on host.

Device work per core (8-way batch data-parallel): 128 matmul tiles of
(27pad32 x 128) @ (27pad32 x 301) in bf16 (rel-err gate is 2e-2; bf16
end-to-end measures 7.3e-3), quad-packed via tile_position (32q, 0), plus
the 9.86 MB/core bf16 output writeback.

Why this pacing is deliberate (measured, do not "optimize" casually):
- Matmul cadence is 403ns/tile (160ns weight-swap drain + 301 cols at the
  1.2 GHz cold PE clock). A pipelined variant (hand-emitted InstMatmult
  with ldweights=True, no quads) reaches a flat 252ns/tile = 32.6us
  compute, BUT with all 8 cores in lockstep the shared HBM write path
  sustains only ~0.20-0.30 GB/us per core; concentrating the writeback
  into the shorter window saturates it and the kernel goes ring-bound
  (measured 54-62us vs this kernel's 50.5-51.5us in the same states).
  This kernel's write demand (9.86MB / 51.6us = 0.19 GB/us) rides just
  under worst-case supply, so the writeback hides behind compute.
- Starting the writeback ring earlier than tile 31 also regresses
  (measured +1.5-4us): the early output writes contend with the SWDGE
  input transfer on HBM and delay the claim-absorbed B chunk.
- PE HAM clock-gate: only full-K (128-row) streaming warms the PE to
  2.4 GHz, but 4x-replicated K on 8 concurrent cores trips chip power
  throttling (util clamped ~55-61%) -- net loss.
- bf16 output is the precision floor (fp8's 2^-4 element error exceeds
  the 2e-2 gate at max-magnitude elements), so output bytes cannot
  shrink further.

Layout: per-core rows are packed 4 tiles at a time into the partition
dim (4 groups of 32 partitions, K padded 27->32 with zeros) so each
quad issues 4 row-group-packed concurrent matmuls (tile_position
(32q, 0) auto-derived from base partitions). PSUM drain alternates
DVE/ACT per tile; the tile -> output-row map (SIGMA) gives each copy
engine a contiguous half of the per-partition rows so every writeback
DMA carries a single sem wait (walrus allows one wait per instruction,
and HWDGE has 8 sem lanes: 1 input + 7 output DMAs).
"""

import numpy as np

# DMP hyperparameters fixed by Net.__init__ (hardcoded per problem spec)
N = 25
DOF = 2
DT = 0.01
TAU = 3.0
A_X = 2.0
A_Z = 48.0
B_Z = A_Z / 4.0
T = 301                    # time steps
BATCH = 65536
PARAM_DIM = DOF * (N + 2)  # 54
NCORES = 8

ROWS = BATCH * DOF         # 131072 (B*DOF rows)
RPC = ROWS // NCORES       # 16384 rows per core
TILES = RPC // 128         # 128 tiles of 128 rows per core
QUAD_COLS = RPC // 4       # 4096: vt free dim (4 tiles packed in partition dim)
G = 8                      # tiles per output staging block
NBLK = TILES // G          # 16 output blocks per core
# PSUM->SBUF copies alternate DVE/ACT per TILE so both engines drain PSUM
# concurrently (per-BLOCK alternation serializes: with 7 PSUM slots < 8
# tiles/block, the pipeline always advances at single-engine copy rate and
# the PE HAM clock-gate never warms past 1.2 GHz). Each output DMA must
# wait on ONE engine's sem (walrus: one wait per instruction), so the tile
# -> output-row map sends even (DVE) tiles to rows m in [0,64) and odd
# (ACT) tiles to m in [64,128): every DMA covers one engine's contiguous
# rows. SIGMA[tile] = m; the host packs lhsT accordingly.
SIGMA = [ti // 2 if ti % 2 == 0 else 64 + ti // 2 for ti in range(TILES)]
# (after_tile, m_start, m_count): 7 DMAs (input DMA takes the 8th HWDGE
# sem lane), emitted in completion order, all rows of one engine each.
DMA_PLAN = (
    (31, 0, 16),
    (55, 64, 28),
    (79, 16, 24),
    (103, 92, 24),
    (119, 40, 20),
    (127, 60, 4),
    (127, 116, 12),
)
assert sorted(m for _, m0, n in DMA_PLAN for m in range(m0, m0 + n)) == list(
    range(128)
)
assert len(DMA_PLAN) + 1 <= 8
for _at, _m0, _n in DMA_PLAN:
    assert len({0 if m < 64 else 1 for m in range(_m0, _m0 + _n)}) == 1
    _last = max(ti for ti in range(TILES) if _m0 <= SIGMA[ti] < _m0 + _n)
    assert _last <= _at, (_at, _m0, _n, _last)

# float32r (1 cyc/row) fails walrus's matmul ISA check in this build for
# every variant probed (K=128/K=32, with/without tile_position) — stay fp32.
USE_F32R = False

# rel-err gate is 2e-2; bf16 in/out measures 7.3e-3 end-to-end (host sim),
# and halves both the 19.7 MB/core output writeback and the input DMA.
USE_BF16 = True

A_QUADS = 8                # quads in the head input chunk (HWDGE)


# ----------------------------------------------------------------------------
# Host-side constant build (exact, float64)
# ----------------------------------------------------------------------------
_const_cache = {}


def _build_constants(c=None, sigma2=None):
    """Return hc (128, 301) float32: rows 32q+p hold
    p==0 -> A, p==1 -> B, p==2+n -> H[:, n], rows 27..31 of each group zero."""
    if c is None:
        c = np.exp(-A_X * np.linspace(0.0, 1.0, N))
    if sigma2 is None:
        sigma2 = (N ** 1.5) / c / A_X
    c = np.asarray(c, np.float64)
    sigma2 = np.asarray(sigma2, np.float64)
    key = (c.tobytes(), sigma2.tobytes())
    if key in _const_cache:
        return _const_cache[key]

    k = DT / TAU
    M = np.array([[1.0, k], [-A_Z * B_Z * k, 1.0 - A_Z * k]])
    P = np.zeros(T + 1)
    Q = np.zeros(T + 1)
    Mn = np.eye(2)
    for n in range(T + 1):
        P[n] = Mn[0, 0]
        Q[n] = Mn[0, 1]
        Mn = Mn @ M

    decay = 1.0 - A_X * DT / TAU
    cx = decay ** np.arange(1, T + 1)                        # cx_1..cx_T
    psi = np.exp(-0.5 * (cx[:, None] - c[None, :]) ** 2 / sigma2[None, :])
    g = psi * (cx / psi.sum(1))[:, None]                     # (T, N)

    A = P[1:T + 1]
    B = k * A_Z * B_Z * np.cumsum(Q[0:T])
    # H[i] = k * sum_{m<=i} Q[i-m] g[m]  -- lower-triangular Toeplitz matvec
    ii = np.arange(T)[:, None]
    mm = np.arange(T)[None, :]
    L = np.where(ii >= mm, Q[np.clip(ii - mm, 0, T)], 0.0)   # (T, T)
    H = k * (L @ g)                                          # (T, N)

    hfull = np.zeros((32, T), np.float32)
    hfull[0] = A.astype(np.float32)
    hfull[1] = B.astype(np.float32)
    hfull[2:2 + N] = H.T.astype(np.float32)
    hc = np.tile(hfull, (4, 1))                              # (128, T)
    _const_cache[key] = hc
    return hc


def _pack_inputs(x, c, sigma2, scale):
    """Build per-core vt arrays (128, 4096) + shared hc (128, 301)."""
    x = np.asarray(x, np.float32)
    if scale is None:
        scale = np.ones(PARAM_DIM, np.float32)
    p = (x * np.asarray(scale, np.float32)).reshape(ROWS, N + 2)
    y0 = p[:, 0]
    goal = p[:, 1]
    u = goal - y0
    v = np.empty((ROWS, N + 2), np.float32)
    v[:, 0] = y0
    v[:, 1] = goal
    v[:, 2:] = p[:, 2:] * u[:, None]

    hc = _build_constants(c, sigma2)

    if USE_BF16:
        from ml_dtypes import bfloat16
        io_dt = bfloat16
    else:
        io_dt = np.float32

    sig = np.asarray(SIGMA, np.int64).reshape(TILES // 4, 4)   # [j, q] -> m
    vts = []
    for i in range(NCORES):
        vc = v[RPC * i:RPC * (i + 1)]                 # (16384, 27)
        # Local row = 128*f + m (f = out partition). Tile ti = 4j+q computes
        # m = SIGMA[ti] so each copy engine owns a contiguous half of the
        # per-partition rows (single-sem-wait writeback DMAs).
        vc3 = vc.reshape(128, 128, N + 2)             # [f, m, p]
        v4 = vc3[:, sig, :].transpose(2, 3, 1, 0)     # [q, p, j, f]
        vp = np.zeros((4, 32, TILES // 4, 128), np.float32)
        vp[:, :N + 2] = v4
        vts.append(np.ascontiguousarray(vp.reshape(128, QUAD_COLS).astype(io_dt)))
    return vts, hc.astype(io_dt)


# ----------------------------------------------------------------------------
# Bass kernel
# ----------------------------------------------------------------------------
_nc_cache = []


def _build_bass():
    if _nc_cache:
        return _nc_cache[0]
    import concourse.bass as bass
    import concourse.mybir as mybir
    from concourse import tile
    import bass_rust
    from concourse.vector_clock import ScopedClock

    class SplitDrainTileContext(tile.TileContext):
        """This walrus build allows a single sync wait per instruction, but
        TileContext's kernel-tail drain carries one wait per live sem lane.
        Split the extras onto standalone single-wait SP nops (same stream, so
        all waits still complete before the barrier + sem clearing)."""

        def _drain_and_barrier(self, tick_clock, wait_clock):
            nc = self.nc
            drain_inst = nc.sync.drain()
            wait_clock.add_sem_waits(
                drain_inst.ins, ScopedClock({None: tick_clock.global_clock})
            )
            si = drain_inst.ins.sync_info
            waits = list(si.on_wait) if si is not None else []
            if len(waits) > 1:
                drain_inst.ins.sync_info = bass_rust.SyncInfo(
                    on_wait=[waits[0]], on_update=list(si.on_update)
                )
                for w in waits[1:]:
                    n = nc.sync.nop(nofuse=True)
                    n.ins.sync_info = bass_rust.SyncInfo(
                        on_wait=[w], on_update=[]
                    )
            nc.all_engine_barrier()
            assert self.sems is not None
            popped = nc._tile_sem_poison_stack.pop()
            assert popped is self._sem_poison
            nc.clear_and_free_semaphores(list(self.sems.allocated().values()))
            nc.all_engine_barrier()

    f32 = mybir.dt.float32
    fmm = mybir.dt.float32r if USE_F32R else f32
    if USE_BF16:
        fmm = mybir.dt.bfloat16
    fout = mybir.dt.bfloat16 if USE_BF16 else f32
    nc = bass.Bass()
    # Input split: a small head chunk (first A_QUADS quads + the 301 constant
    # columns) on HWDGE so compute starts after ~2us, the rest on SWDGE in
    # parallel. Single tensors per chunk keep every matmul at one sync wait
    # (walrus allows a single S3_LW wait slot per self-loading matmul).
    va_d = nc.dram_tensor("va", [128, 128 * A_QUADS + T], fmm, kind="ExternalInput")
    vb_d = nc.dram_tensor(
        "vb", [128, QUAD_COLS - 128 * A_QUADS], fmm, kind="ExternalInput"
    )
    out_d = nc.dram_tensor("out", [RPC, T], fout, kind="ExternalOutput")

    with SplitDrainTileContext(nc) as tc:
        with (
            tc.tile_pool(name="vtp", bufs=1) as vtp,
            tc.tile_pool(name="stage", bufs=1) as stagep,
            tc.tile_pool(name="psum", bufs=7, space="PSUM") as psump,
            tc.tile_pool(name="clm", bufs=1, space="PSUM") as clmp,
        ):
            vtsA = vtp.tile([128, 128 * A_QUADS + T], fmm, tag="vtsA")
            vtsB = vtp.tile([128, QUAD_COLS - 128 * A_QUADS], fmm, tag="vtsB")
            nc.sync.dma_start(vtsA[:], va_d[:])
            nc.gpsimd.dma_start(vtsB[:], vb_d[:])
            hrep = vtsA[:, 128 * A_QUADS:128 * A_QUADS + T]

            def lhsT(j, q):
                if j < A_QUADS:
                    return vtsA[32 * q:32 * q + 32, 128 * j:128 * (j + 1)]
                jb = j - A_QUADS
                return vtsB[32 * q:32 * q + 32, 128 * jb:128 * (jb + 1)]

            # One persistent staging buffer for the whole per-core output.
            # No slot recycling -> no release waits, so every copy carries
            # only its PE wait (walrus allows a single sync wait per DVE /
            # matmul instruction).
            stage = stagep.tile([128, TILES, T], fout)

            # local row = 128*p + m: per-partition output is linear in HBM,
            # so writeback DMAs are long contiguous bursts per partition.
            out_lin = out_d.rearrange("(p m) t -> p m t", p=128, m=TILES)

            dma_after = {}
            for at, m0, n in DMA_PLAN:
                dma_after.setdefault(at, []).append((m0, n))

            for ti in range(TILES):
                j, q = ti // 4, ti % 4
                if ti == 4 * (A_QUADS // 2):
                    # Tiny claim matmul: absorbs the B-chunk DMA wait on
                    # the PE clock so later B matmuls carry only their
                    # psum-release wait. Placed mid-A so PE never stalls.
                    cps = clmp.tile([128, 8], f32)
                    nc.tensor.matmul(
                        cps[:1, :1],
                        vtsB[:1, :1],
                        vtsB[:1, 1:2],
                        start=True,
                        stop=True,
                        tile_position=(0, 0),
                    )
                ps = psump.tile([128, T], f32)
                nc.tensor.matmul(
                    ps[:],
                    lhsT(j, q),
                    hrep[32 * q:32 * q + 32, :],
                    start=True,
                    stop=True,
                    tile_position=(32 * q, 0),
                )
                if ti % 2 == 0:
                    nc.vector.tensor_copy(stage[:, SIGMA[ti], :], ps[:])
                else:
                    nc.scalar.copy(stage[:, SIGMA[ti], :], ps[:])
                for m0, n in dma_after.get(ti, ()):
                    # All output groups on the single SP HWDGE ring: splitting
                    # across SWDGE (81.2us) or the ACT HWDGE ring (80.5us)
                    # measured strictly worse than one saturated ring (74.0us)
                    # — interleaved rings fragment the HBM write stream.
                    nc.sync.dma_start(
                        out_lin[:, m0:m0 + n, :], stage[:, m0:m0 + n, :]
                    )

    _nc_cache.append(nc)
    return nc


def _run(in_maps, trace=False):
    from concourse.bass_utils import run_bass_kernel_spmd

    nc = _build_bass()
    return run_bass_kernel_spmd(nc, in_maps, list(range(NCORES)), trace=trace)


def kernel(x, c=None, sigma2=None, scale=None, _trace=False):
    vts, hc = _pack_inputs(x, c, sigma2, scale)
    acols = 128 * A_QUADS
    in_maps = [
        {
            "va": np.ascontiguousarray(
                np.concatenate([vts[i][:, :acols], hc], axis=1)
            ),
            "vb": np.ascontiguousarray(vts[i][:, acols:]),
        }
        for i in range(NCORES)
    ]
    res = _run(in_maps, trace=_trace)
    out = np.concatenate(
        [np.asarray(res.results[i]["out"], np.float32) for i in range(NCORES)],
        axis=0,
    )
    out = out.reshape(BATCH, DOF, T)
    if _trace:
        return out, res
    return out



# revision 57
# speedup vs baseline: 1.0059x; 1.0059x over previous
"""Trainium2 Bass kernel for the DMP-rollout Net (nn_Net_60567628808344).

Math
----
The reference integrates, per row r of p = (x*scale).reshape(-1, 27):
    y0 = p[:,0], goal = p[:,1], w = p[:,2:]
    cx_j = (1 - A_X*DT/TAU)^j                     (data independent)
    psi_j = exp(-0.5 (cx_j - c)^2 / sigma2)       (data independent)
    state update is LINEAR:  s_j = M s_{j-1} + [0; k*(Az*Bz*goal + F_j)]
    with F_j = (w @ psi_j) * cx_j * (goal-y0) / sum(psi_j)
So the whole 301-step rollout collapses to a closed form
    out[r, i] = A[i]*y0_r + B[i]*goal_r + (goal_r - y0_r) * (w_r @ H[i, :])
with constant A,B (301,), H (301, 25) precomputed in float64 on host.

Device work per core (8-way batch data-parallel): 128 matmul tiles of
(27pad32 x 128) @ (27pad32 x 301) in bf16 (rel-err gate is 2e-2; bf16
end-to-end measures 7.3e-3), quad-packed via tile_position (32q, 0), plus
the 9.86 MB/core bf16 output writeback.

Why this pacing is deliberate (measured, do not "optimize" casually):
- Matmul cadence is 403ns/tile (160ns weight-swap drain + 301 cols at the
  1.2 GHz cold PE clock). A pipelined variant (hand-emitted InstMatmult
  with ldweights=True, no quads) reaches a flat 252ns/tile = 32.6us
  compute, BUT with all 8 cores in lockstep the shared HBM write path
  sustains only ~0.20-0.30 GB/us per core; concentrating the writeback
  into the shorter window saturates it and the kernel goes ring-bound
  (measured 54-62us vs this kernel's 50.5-51.5us in the same states).
  This kernel's write demand (9.86MB / 51.6us = 0.19 GB/us) rides just
  under worst-case supply, so the writeback hides behind compute.
- Starting the writeback ring earlier than tile 31 also regresses
  (measured +1.5-4us): the early output writes contend with the SWDGE
  input transfer on HBM and delay the claim-absorbed B chunk.
- PE HAM clock-gate: only full-K (128-row) streaming warms the PE to
  2.4 GHz, but 4x-replicated K on 8 concurrent cores trips chip power
  throttling (util clamped ~55-61%) -- net loss.
- bf16 output is the precision floor (fp8's 2^-4 element error exceeds
  the 2e-2 gate at max-magnitude elements), so output bytes cannot
  shrink further.

Layout: per-core rows are packed 4 tiles at a time into the partition
dim (4 groups of 32 partitions, K padded 27->32 with zeros) so each
quad issues 4 row-group-packed concurrent matmuls (tile_position
(32q, 0) auto-derived from base partitions). PSUM drain alternates
DVE/ACT per tile; the tile -> output-row map (SIGMA) gives each copy
engine a contiguous half of the per-partition rows so every writeback
DMA carries a single sem wait (walrus allows one wait per instruction,
and HWDGE has 8 sem lanes: 1 input + 7 output DMAs).
"""

import numpy as np

# DMP hyperparameters fixed by Net.__init__ (hardcoded per problem spec)
N = 25
DOF = 2
DT = 0.01
TAU = 3.0
A_X = 2.0
A_Z = 48.0
B_Z = A_Z / 4.0
T = 301                    # time steps
BATCH = 65536
PARAM_DIM = DOF * (N + 2)  # 54
NCORES = 8

ROWS = BATCH * DOF         # 131072 (B*DOF rows)
RPC = ROWS // NCORES       # 16384 rows per core
TILES = RPC // 128         # 128 tiles of 128 rows per core
QUAD_COLS = RPC // 4       # 4096: vt free dim (4 tiles packed in partition dim)
G = 8                      # tiles per output staging block
NBLK = TILES // G          # 16 output blocks per core
# PSUM->SBUF copies alternate DVE/ACT per TILE so both engines drain PSUM
# concurrently (per-BLOCK alternation serializes: with 7 PSUM slots < 8
# tiles/block, the pipeline always advances at single-engine copy rate and
# the PE HAM clock-gate never warms past 1.2 GHz). Each output DMA must
# wait on ONE engine's sem (walrus: one wait per instruction), so the tile
# -> output-row map sends even (DVE) tiles to rows m in [0,64) and odd
# (ACT) tiles to m in [64,128): every DMA covers one engine's contiguous
# rows. SIGMA[tile] = m; the host packs lhsT accordingly.
SIGMA = [ti // 2 if ti % 2 == 0 else 64 + ti // 2 for ti in range(TILES)]
# (after_tile, m_start, m_count): 7 DMAs (input DMA takes the 8th HWDGE
# sem lane), emitted in completion order, all rows of one engine each.
DMA_PLAN = (
    (31, 0, 16),
    (55, 64, 28),
    (79, 16, 24),
    (103, 92, 24),
    (119, 40, 20),
    (127, 60, 4),
    (127, 116, 12),
)
assert sorted(m for _, m0, n in DMA_PLAN for m in range(m0, m0 + n)) == list(
    range(128)
)
assert len(DMA_PLAN) + 1 <= 8
for _at, _m0, _n in DMA_PLAN:
    assert len({0 if m < 64 else 1 for m in range(_m0, _m0 + _n)}) == 1
    _last = max(ti for ti in range(TILES) if _m0 <= SIGMA[ti] < _m0 + _n)
    assert _last <= _at, (_at, _m0, _n, _last)

# float32r (1 cyc/row) fails walrus's matmul ISA check in this build for
# every variant probed (K=128/K=32, with/without tile_position) — stay fp32.
USE_F32R = False

# rel-err gate is 2e-2; bf16 in/out measures 7.3e-3 end-to-end (host sim),
# and halves both the 19.7 MB/core output writeback and the input DMA.
USE_BF16 = True

A_QUADS = 8                # quads in the head input chunk (HWDGE)


# ----------------------------------------------------------------------------
# Host-side constant build (exact, float64)
# ----------------------------------------------------------------------------
_const_cache = {}


def _build_constants(c=None, sigma2=None):
    """Return hc (128, 301) float32: rows 32q+p hold
    p==0 -> A, p==1 -> B, p==2+n -> H[:, n], rows 27..31 of each group zero."""
    if c is None:
        c = np.exp(-A_X * np.linspace(0.0, 1.0, N))
    if sigma2 is None:
        sigma2 = (N ** 1.5) / c / A_X
    c = np.asarray(c, np.float64)
    sigma2 = np.asarray(sigma2, np.float64)
    key = (c.tobytes(), sigma2.tobytes())
    if key in _const_cache:
        return _const_cache[key]

    k = DT / TAU
    M = np.array([[1.0, k], [-A_Z * B_Z * k, 1.0 - A_Z * k]])
    P = np.zeros(T + 1)
    Q = np.zeros(T + 1)
    Mn = np.eye(2)
    for n in range(T + 1):
        P[n] = Mn[0, 0]
        Q[n] = Mn[0, 1]
        Mn = Mn @ M

    decay = 1.0 - A_X * DT / TAU
    cx = decay ** np.arange(1, T + 1)                        # cx_1..cx_T
    psi = np.exp(-0.5 * (cx[:, None] - c[None, :]) ** 2 / sigma2[None, :])
    g = psi * (cx / psi.sum(1))[:, None]                     # (T, N)

    A = P[1:T + 1]
    B = k * A_Z * B_Z * np.cumsum(Q[0:T])
    # H[i] = k * sum_{m<=i} Q[i-m] g[m]  -- lower-triangular Toeplitz matvec
    ii = np.arange(T)[:, None]
    mm = np.arange(T)[None, :]
    L = np.where(ii >= mm, Q[np.clip(ii - mm, 0, T)], 0.0)   # (T, T)
    H = k * (L @ g)                                          # (T, N)

    hfull = np.zeros((32, T), np.float32)
    hfull[0] = A.astype(np.float32)
    hfull[1] = B.astype(np.float32)
    hfull[2:2 + N] = H.T.astype(np.float32)
    hc = np.tile(hfull, (4, 1))                              # (128, T)
    _const_cache[key] = hc
    return hc


def _pack_inputs(x, c, sigma2, scale):
    """Build per-core vt arrays (128, 4096) + shared hc (128, 301)."""
    x = np.asarray(x, np.float32)
    if scale is None:
        scale = np.ones(PARAM_DIM, np.float32)
    p = (x * np.asarray(scale, np.float32)).reshape(ROWS, N + 2)
    y0 = p[:, 0]
    goal = p[:, 1]
    u = goal - y0
    v = np.empty((ROWS, N + 2), np.float32)
    v[:, 0] = y0
    v[:, 1] = goal
    v[:, 2:] = p[:, 2:] * u[:, None]

    hc = _build_constants(c, sigma2)

    if USE_BF16:
        from ml_dtypes import bfloat16
        io_dt = bfloat16
    else:
        io_dt = np.float32

    sig = np.asarray(SIGMA, np.int64).reshape(TILES // 4, 4)   # [j, q] -> m
    vts = []
    for i in range(NCORES):
        vc = v[RPC * i:RPC * (i + 1)]                 # (16384, 27)
        # Local row = 128*f + m (f = out partition). Tile ti = 4j+q computes
        # m = SIGMA[ti] so each copy engine owns a contiguous half of the
        # per-partition rows (single-sem-wait writeback DMAs).
        vc3 = vc.reshape(128, 128, N + 2)             # [f, m, p]
        v4 = vc3[:, sig, :].transpose(2, 3, 1, 0)     # [q, p, j, f]
        vp = np.zeros((4, 32, TILES // 4, 128), np.float32)
        vp[:, :N + 2] = v4
        vts.append(np.ascontiguousarray(vp.reshape(128, QUAD_COLS).astype(io_dt)))
    return vts, hc.astype(io_dt)


# ----------------------------------------------------------------------------
# Bass kernel
# ----------------------------------------------------------------------------
_nc_cache = []


def _build_bass():
    if _nc_cache:
        return _nc_cache[0]
    import concourse.bass as bass
    import concourse.mybir as mybir
    from concourse import tile
    import bass_rust
    from concourse.vector_clock import ScopedClock

    class SplitDrainTileContext(tile.TileContext):
        """This walrus build allows a single sync wait per instruction, but
        TileContext's kernel-tail drain carries one wait per live sem lane.
        Split the extras onto standalone single-wait SP nops (same stream, so
        all waits still complete before the barrier + sem clearing)."""

        def _drain_and_barrier(self, tick_clock, wait_clock):
            nc = self.nc
            drain_inst = nc.sync.drain()
            wait_clock.add_sem_waits(
                drain_inst.ins, ScopedClock({None: tick_clock.global_clock})
            )
            si = drain_inst.ins.sync_info
            waits = list(si.on_wait) if si is not None else []
            if len(waits) > 1:
                drain_inst.ins.sync_info = bass_rust.SyncInfo(
                    on_wait=[waits[0]], on_update=list(si.on_update)
                )
                for w in waits[1:]:
                    n = nc.sync.nop(nofuse=True)
                    n.ins.sync_info = bass_rust.SyncInfo(
                        on_wait=[w], on_update=[]
                    )
            nc.all_engine_barrier()
            assert self.sems is not None
            popped = nc._tile_sem_poison_stack.pop()
            assert popped is self._sem_poison
            nc.clear_and_free_semaphores(list(self.sems.allocated().values()))
            nc.all_engine_barrier()

    f32 = mybir.dt.float32
    fmm = mybir.dt.float32r if USE_F32R else f32
    if USE_BF16:
        fmm = mybir.dt.bfloat16
    fout = mybir.dt.bfloat16 if USE_BF16 else f32
    nc = bass.Bass()
    # Input split: a small head chunk (first A_QUADS quads + the 301 constant
    # columns) on HWDGE so compute starts after ~2us, the rest on SWDGE in
    # parallel. Single tensors per chunk keep every matmul at one sync wait
    # (walrus allows a single S3_LW wait slot per self-loading matmul).
    va_d = nc.dram_tensor("va", [128, 128 * A_QUADS + T], fmm, kind="ExternalInput")
    vb_d = nc.dram_tensor(
        "vb", [128, QUAD_COLS - 128 * A_QUADS], fmm, kind="ExternalInput"
    )
    out_d = nc.dram_tensor("out", [RPC, T], fout, kind="ExternalOutput")

    with SplitDrainTileContext(nc) as tc:
        with (
            tc.tile_pool(name="vtp", bufs=1) as vtp,
            tc.tile_pool(name="stage", bufs=1) as stagep,
            tc.tile_pool(name="psum", bufs=7, space="PSUM") as psump,
            tc.tile_pool(name="clm", bufs=1, space="PSUM") as clmp,
        ):
            vtsA = vtp.tile([128, 128 * A_QUADS + T], fmm, tag="vtsA")
            vtsB = vtp.tile([128, QUAD_COLS - 128 * A_QUADS], fmm, tag="vtsB")
            nc.sync.dma_start(vtsA[:], va_d[:])
            nc.gpsimd.dma_start(vtsB[:], vb_d[:])
            hrep = vtsA[:, 128 * A_QUADS:128 * A_QUADS + T]

            def lhsT(j, q):
                if j < A_QUADS:
                    return vtsA[32 * q:32 * q + 32, 128 * j:128 * (j + 1)]
                jb = j - A_QUADS
                return vtsB[32 * q:32 * q + 32, 128 * jb:128 * (jb + 1)]

            # One persistent staging buffer for the whole per-core output.
            # No slot recycling -> no release waits, so every copy carries
            # only its PE wait (walrus allows a single sync wait per DVE /
            # matmul instruction).
            stage = stagep.tile([128, TILES, T], fout)

            # local row = 128*p + m: per-partition output is linear in HBM,
            # so writeback DMAs are long contiguous bursts per partition.
            out_lin = out_d.rearrange("(p m) t -> p m t", p=128, m=TILES)

            dma_after = {}
            for at, m0, n in DMA_PLAN:
                dma_after.setdefault(at, []).append((m0, n))

            for ti in range(TILES):
                j, q = ti // 4, ti % 4
                if ti == 4 * A_QUADS - 4:
                    # Tiny claim matmul: absorbs the B-chunk DMA wait on
                    # the PE clock so later B matmuls carry only their
                    # psum-release wait. Placed as late as possible before
                    # the first B consumer (tile 32): the SWDGE transfer
                    # lands ~10.5-12.2us but can slip under HBM read/write
    # contention in slow chip states, so maximize the margin.
                    cps = clmp.tile([128, 8], f32)
                    nc.tensor.matmul(
                        cps[:1, :1],
                        vtsB[:1, :1],
                        vtsB[:1, 1:2],
                        start=True,
                        stop=True,
                        tile_position=(0, 0),
                    )
                ps = psump.tile([128, T], f32)
                nc.tensor.matmul(
                    ps[:],
                    lhsT(j, q),
                    hrep[32 * q:32 * q + 32, :],
                    start=True,
                    stop=True,
                    tile_position=(32 * q, 0),
                )
                if ti % 2 == 0:
                    nc.vector.tensor_copy(stage[:, SIGMA[ti], :], ps[:])
                else:
                    nc.scalar.copy(stage[:, SIGMA[ti], :], ps[:])
                for m0, n in dma_after.get(ti, ()):
                    # All output groups on the single SP HWDGE ring: splitting
                    # across SWDGE (81.2us) or the ACT HWDGE ring (80.5us)
                    # measured strictly worse than one saturated ring (74.0us)
                    # — interleaved rings fragment the HBM write stream.
                    nc.sync.dma_start(
                        out_lin[:, m0:m0 + n, :], stage[:, m0:m0 + n, :]
                    )

    _nc_cache.append(nc)
    return nc


def _run(in_maps, trace=False):
    from concourse.bass_utils import run_bass_kernel_spmd

    nc = _build_bass()
    return run_bass_kernel_spmd(nc, in_maps, list(range(NCORES)), trace=trace)


def kernel(x, c=None, sigma2=None, scale=None, _trace=False):
    vts, hc = _pack_inputs(x, c, sigma2, scale)
    acols = 128 * A_QUADS
    in_maps = [
        {
            "va": np.ascontiguousarray(
                np.concatenate([vts[i][:, :acols], hc], axis=1)
            ),
            "vb": np.ascontiguousarray(vts[i][:, acols:]),
        }
        for i in range(NCORES)
    ]
    res = _run(in_maps, trace=_trace)
    out = np.concatenate(
        [np.asarray(res.results[i]["out"], np.float32) for i in range(NCORES)],
        axis=0,
    )
    out = out.reshape(BATCH, DOF, T)
    if _trace:
        return out, res
    return out

